# revision 17
# baseline (speedup 1.0000x reference)
"""Trainium2 Bass kernel for banded (episodic-memory) attention.

Module computation (B=4, S=4096, D=256, d2=512, band width 64):
    x = states.reshape(B, S, 512)
    q = x @ Wq.T ; k = x @ Wk.T
    scores = q @ k.T / sqrt(512), masked to j in [i-64, i-1]
    w = softmax(scores)  (fully-masked row 0 -> 0)
    retrieved = w @ x
    returns (retrieved.reshape(B,S,256,2), w)

Device strategy (8 cores = 4 batches x 2 sequence halves):
    scores[i,j] = x_i^T (Wq^T Wk) x_j / sqrt(512) = z_i . x_j with
    z_i = (Wq^T Wk)^T x_i -- one fused projection instead of two, and the
    key side of the score matmul is x^T directly.  Per 128-query block the
    live key window is 256 wide (band is 64), so scores are a [128, 256]
    tile with a static additive band mask (built on-device); exp with
    fused row-sum (no max-subtract needed: |s| is a few units at most);
    retrieval contracts the 256-window transposed weights against the
    values.  Unnormalized band + row sums go back to the host, which
    normalizes and scatters into the dense [S, S] output (entries outside
    the band are exactly 0).  Score-path matmuls run in float32r
    (TF32-like, ~1.5e-4 rel err, full PE rate at N>=256); the value path
    runs in bf16.  The left sequence edge (h=0) is handled by zero-padded
    keys: padded columns contribute exp(0)=1 to the row sum, which the
    host subtracts exactly before normalizing.
"""

import sys
from contextlib import ExitStack

if "/opt/trn_rl_repo" not in sys.path:
    sys.path.insert(0, "/opt/trn_rl_repo")

import numpy as np

B, S, D2 = 4, 4096, 512
BAND = 64
SH = S // 2          # 2048 rows per core
CTXP = SH + 2 * BAND  # 2176 padded context rows (= 17 * 128)
NBLK = SH // 128      # 16 query blocks per core
WIN = 256             # key window per query block
NEG = -1e30
# xT column-chunk boundaries: chunk t exactly covers phase-A tile t's needs
XT_CHUNKS = [(0, 576), (576, 1088), (1088, 1600), (1600, 2176)]

_PROGRAMS = {}


def _build_program(reps: int = 1, probe: str | None = None):
    # reps > 1 repeats the whole compute body (same inputs/outputs) inside an
    # on-device For_i loop so a benchmark can difference wall times to
    # isolate per-iteration device time.
    # probe: None (full kernel) | "io" (DMAs only) | "compute" (no DMAs).
    do_io = probe != "compute"
    do_compute = probe != "io"
    import concourse.tile as tile
    from concourse import bacc, mybir
    from concourse.masks import make_identity

    F32 = mybir.dt.float32
    F32R = mybir.dt.float32r
    BF16 = mybir.dt.bfloat16
    Exp = mybir.ActivationFunctionType.Exp
    Copy = mybir.ActivationFunctionType.Copy

    nc = bacc.Bacc("TRN2", target_bir_lowering=False, debug=False)

    xT_d = nc.dram_tensor("xT", [D2, CTXP], F32R, kind="ExternalInput").ap()
    xv_d = nc.dram_tensor("xv", [CTXP, D2], BF16, kind="ExternalInput").ap()
    mT_d = nc.dram_tensor("mT", [D2, D2], F32R, kind="ExternalInput").ap()
    wb_d = nc.dram_tensor("wb", [SH, 192], F32, kind="ExternalOutput").ap()
    rs_d = nc.dram_tensor("rs", [128, NBLK], F32, kind="ExternalOutput").ap()
    ro_d = nc.dram_tensor("ro", [SH, D2], F32, kind="ExternalOutput").ap()

    with tile.TileContext(nc) as tc:
        with (
            tc.tile_pool(name="const", bufs=1) as cp,
            tc.tile_pool(name="spool", bufs=3) as s_pool,
            tc.tile_pool(name="wpool", bufs=3) as w_pool,
            tc.tile_pool(name="wtpool", bufs=3) as wt_pool,
            tc.tile_pool(name="ropool", bufs=3) as ro_pool,
            tc.tile_pool(name="stats", bufs=6) as st_pool,
            tc.tile_pool(name="pa", bufs=2, space="PSUM") as pa,
            tc.tile_pool(name="ps", bufs=2, space="PSUM") as ps,
            tc.tile_pool(name="pt", bufs=2, space="PSUM") as pt,
            tc.tile_pool(name="pr", bufs=2, space="PSUM") as pr,
            ExitStack() as loop_ctx,
        ):
            ident = cp.tile([128, 128], F32)
            make_identity(nc, ident[:])
            # band mask: allowed iff qi <= kj <= qi+63, else NEG (additive)
            ms_sb = cp.tile([128, WIN], F32)
            nc.gpsimd.memset(ms_sb[:], 0.0)
            nc.gpsimd.affine_select(
                out=ms_sb[:], in_=ms_sb[:], compare_op=mybir.AluOpType.is_ge,
                fill=NEG, base=0, pattern=[[1, WIN]], channel_multiplier=-1,
            )
            nc.gpsimd.affine_select(
                out=ms_sb[:], in_=ms_sb[:], compare_op=mybir.AluOpType.is_ge,
                fill=NEG, base=BAND - 1, pattern=[[-1, WIN]], channel_multiplier=1,
            )
            xT_sb = cp.tile([128, 4, CTXP], F32R)
            xv_sb = cp.tile([128, CTXP // 128, D2], BF16)
            mT_sb = cp.tile([128, 4, D2], F32R)
            zt_sb = cp.tile([128, 4, SH], F32R)
            rs_sb = cp.tile([128, NBLK], F32)

            if not do_io:
                # compute-only probe: one-time zero-init so matmul inputs
                # have writers (outside the bench loop)
                for t in (xT_sb, mT_sb):
                    nc.gpsimd.memset(t[:].bitcast(F32), 0.0)
                nc.gpsimd.memset(xv_sb[:], 0.0)

            if reps > 1:
                loop_ctx.enter_context(tc.For_i(0, reps, 1, staggered_reset=True))

            # ---- input loads (inside the bench loop so per-iteration time
            # includes the real input DMA cost) ----
            if do_io:
                for dc in range(4):
                    nc.sync.dma_start(
                        mT_sb[:, dc, :], mT_d[dc * 128 : (dc + 1) * 128, :]
                    )
                for dc in range(4):
                    for c0, c1 in XT_CHUNKS:
                        nc.sync.dma_start(
                            xT_sb[:, dc, c0:c1],
                            xT_d[dc * 128 : (dc + 1) * 128, c0:c1],
                        )
                for t in range(CTXP // 128):
                    nc.sync.dma_start(xv_sb[:, t, :], xv_d[t * 128 : (t + 1) * 128, :])

            if do_compute:
                # ---- phase A: zT = (Wq^T Wk / sqrt(d2))^T-proj of queries ----
                for st in range(4):
                    for ec in range(4):
                        pz = pa.tile([128, 512], F32, tag="pz", name="pz")
                        for dc in range(4):
                            nc.tensor.matmul(
                                pz[:],
                                mT_sb[:, dc, ec * 128 : (ec + 1) * 128],
                                xT_sb[:, dc, BAND + st * 512 : BAND + (st + 1) * 512],
                                start=(dc == 0),
                                stop=(dc == 3),
                            )
                        dst = zt_sb[:, ec, st * 512 : (st + 1) * 512]
                        if (st * 4 + ec) % 2 == 0:
                            nc.scalar.copy(dst, pz[:])
                        else:
                            nc.vector.tensor_copy(dst, pz[:])

            # ---- phase B: per 128-query block ----
            for p in range(NBLK):
                if do_compute:
                    sps = ps.tile([128, WIN], F32, tag="sps", name="sps")
                    for cc in range(4):
                        nc.tensor.matmul(
                            sps[:],
                            zt_sb[:, cc, p * 128 : (p + 1) * 128],
                            xT_sb[:, cc, p * 128 : p * 128 + WIN],
                            start=(cc == 0),
                            stop=(cc == 3),
                        )
                    s_sb = s_pool.tile([128, WIN], F32, tag="s", name="s_sb")
                    nc.vector.tensor_add(s_sb[:], sps[:], ms_sb[:])
                    w_sb = w_pool.tile([128, WIN], F32, tag="w", name="w_sb")
                    nc.scalar.activation(
                        w_sb[:], s_sb[:], Exp, accum_out=rs_sb[:, p : p + 1]
                    )
                if do_io:
                    wb_src = (
                        w_sb[:, 0:192]
                        if do_compute
                        else xT_sb[:, 0, 0:192].bitcast(F32)
                    )
                    nc.sync.dma_start(wb_d[p * 128 : (p + 1) * 128, :], wb_src)

                if do_compute:
                    radd = st_pool.tile([128, 1], F32, tag="radd", name="radd")
                    nc.vector.tensor_scalar_add(radd[:], rs_sb[:, p : p + 1], 1e-30)
                    rinv = st_pool.tile([128, 1], F32, tag="rinv", name="rinv")
                    nc.vector.reciprocal(rinv[:], radd[:])

                    tps = pt.tile([128, 2, 128], F32, tag="tps", name="tps")
                    nc.tensor.transpose(tps[:, 0, :], w_sb[:, 0:128], ident[:])
                    nc.tensor.transpose(tps[:, 1, :], w_sb[:, 128:256], ident[:])
                    wt_sb = wt_pool.tile([128, 2, 128], BF16, tag="wt", name="wt_sb")
                    nc.vector.tensor_copy(wt_sb[:], tps[:])

                    rps = pr.tile([128, D2], F32, tag="rps", name="rps")
                    nc.tensor.matmul(
                        rps[:], wt_sb[:, 0, :], xv_sb[:, p, :], start=True, stop=False
                    )
                    nc.tensor.matmul(
                        rps[:],
                        wt_sb[:, 1, :],
                        xv_sb[:, p + 1, :],
                        start=False,
                        stop=True,
                    )
                    ro_sb = ro_pool.tile([128, D2], F32, tag="ro", name="ro_sb")
                    nc.scalar.activation(ro_sb[:], rps[:], Copy, scale=rinv[:])
                if do_io:
                    ro_src = (
                        ro_sb[:] if do_compute else xT_sb[:, 0, 0:D2].bitcast(F32)
                    )
                    nc.sync.dma_start(ro_d[p * 128 : (p + 1) * 128, :], ro_src)

            if do_io:
                rs_src = (
                    rs_sb[:] if do_compute else xT_sb[:, 0, 0:NBLK].bitcast(F32)
                )
                nc.sync.dma_start(rs_d, rs_src)

    nc.compile()
    return nc


def _get_program(reps: int = 1, probe: str | None = None):
    key = (reps, probe)
    if key not in _PROGRAMS:
        _PROGRAMS[key] = _build_program(reps, probe)
    return _PROGRAMS[key]


def _make_in_maps(states, Wq, Wk):
    import ml_dtypes

    x = np.ascontiguousarray(states.reshape(B, S, D2), dtype=np.float32)
    scale = np.float64(D2) ** -0.5
    mT = ((Wq.T.astype(np.float64) @ Wk.astype(np.float64)) * scale).astype(np.float32)

    in_maps = []
    for c in range(8):
        b, h = c // 2, c % 2
        s0 = h * SH
        xpad = np.zeros((S + 2 * BAND, D2), dtype=np.float32)
        xpad[BAND : BAND + S] = x[b]
        xv = np.ascontiguousarray(xpad[s0 : s0 + CTXP])
        xT = np.ascontiguousarray(xv.T)
        in_maps.append(
            {
                "xT": xT,
                "xv": xv.astype(ml_dtypes.bfloat16),
                "mT": mT,
            }
        )
    return in_maps


def _assemble(res):
    retrieved = np.empty((B, S, D2), dtype=np.float32)
    w = np.zeros((B, S, S), dtype=np.float32)
    edge = (BAND - np.arange(BAND)).astype(np.float32)  # spurious exp(0) count
    for c in range(8):
        b, h = c // 2, c % 2
        s0 = h * SH
        r = res[c]
        ro = r["ro"]
        rs = r["rs"].T.reshape(SH).astype(np.float32)  # [qi, p] -> row p*128+qi
        rs_used = rs.copy()
        if h == 0:
            # zero-padded keys contributed exp(0)=1 each to rows 0..63
            rs_used[:BAND] = rs[:BAND] - edge
            fix = rs[:BAND] / np.maximum(rs_used[:BAND], 1e-30)
            ro = ro.copy()
            ro[:BAND] *= fix[:, None]
        retrieved[b, s0 : s0 + SH] = ro
        wn = r["wb"] / np.maximum(rs_used, 1e-30)[:, None]
        for p in range(NBLK):
            g0 = s0 + p * 128
            c0 = g0 - BAND
            blk = wn[p * 128 : (p + 1) * 128]
            if c0 < 0:
                w[b, g0 : g0 + 128, 0 : c0 + 192] = blk[:, -c0:]
            else:
                w[b, g0 : g0 + 128, c0 : c0 + 192] = blk
    retrieved[:, 0, :] = 0.0
    w[:, 0, :] = 0.0
    return retrieved.reshape(B, S, D2 // 2, 2), w


def kernel(states: np.ndarray, Wq: np.ndarray, Wk: np.ndarray):
    from concourse.bass_utils import run_bass_kernel_spmd

    in_maps = _make_in_maps(states, Wq, Wk)
    nc = _get_program()
    res = run_bass_kernel_spmd(nc, in_maps, core_ids=list(range(8))).results
    return _assemble(res)


# revision 20
# speedup vs baseline: 1.7563x; 1.7563x over previous
"""Trainium2 Bass kernel for banded (episodic-memory) attention.

Module computation (B=4, S=4096, D=256, d2=512, band width 64):
    x = states.reshape(B, S, 512)
    q = x @ Wq.T ; k = x @ Wk.T
    scores = q @ k.T / sqrt(512), masked to j in [i-64, i-1]
    w = softmax(scores)  (fully-masked row 0 -> 0)
    retrieved = w @ x
    returns (retrieved.reshape(B,S,256,2), w)

Device strategy (8 cores = 4 batches x 2 sequence halves):
    scores[i,j] = x_i^T (Wq^T Wk) x_j / sqrt(512) = z_i . x_j with
    z_i = (Wq^T Wk)^T x_i -- one fused projection instead of two, and the
    key side of the score matmul is x^T directly.  Per 128-query block the
    live key window is 256 wide (band is 64), so scores are a [128, 256]
    tile with a static additive band mask (built on-device); exp with
    fused row-sum (no max-subtract needed: |s| is a few units at most);
    retrieval contracts the 256-window transposed weights against the
    values.  Unnormalized band + row sums go back to the host, which
    normalizes and scatters into the dense [S, S] output (entries outside
    the band are exactly 0).  Score-path matmuls run in float32r
    (TF32-like, ~1.5e-4 rel err, full PE rate at N>=256); the value path
    runs in bf16.  The left sequence edge (h=0) is handled by zero-padded
    keys: padded columns contribute exp(0)=1 to the row sum, which the
    host subtracts exactly before normalizing.
"""

import sys
from contextlib import ExitStack

if "/opt/trn_rl_repo" not in sys.path:
    sys.path.insert(0, "/opt/trn_rl_repo")

import numpy as np

B, S, D2 = 4, 4096, 512
BAND = 64
SH = S // 2          # 2048 rows per core
CTXP = SH + 2 * BAND  # 2176 padded context rows (= 17 * 128)
NBLK = SH // 128      # 16 query blocks per core
WIN = 192             # key window per query block (banded: 64 + 128)
NEG = -1e30
# xT column-chunk boundaries: chunk t exactly covers phase-A tile t's needs
XT_CHUNKS = [(0, 576), (576, 1088), (1088, 1600), (1600, 2176)]

_PROGRAMS = {}


def _build_program(reps: int = 1, probe: str | None = None):
    # reps > 1 repeats the whole compute body (same inputs/outputs) inside an
    # on-device For_i loop so a benchmark can difference wall times to
    # isolate per-iteration device time.
    # probe: None (full kernel) | "io" (DMAs only) | "compute" (no DMAs).
    do_io = probe != "compute"
    do_compute = probe != "io"
    import concourse.tile as tile
    from concourse import bacc, mybir
    from concourse.masks import make_identity

    F32 = mybir.dt.float32
    F32R = mybir.dt.float32r
    F16 = mybir.dt.float16
    Exp = mybir.ActivationFunctionType.Exp
    Copy = mybir.ActivationFunctionType.Copy

    nc = bacc.Bacc("TRN2", target_bir_lowering=False, debug=False)

    xT_d = nc.dram_tensor("xT", [D2, CTXP], F16, kind="ExternalInput").ap()
    xv_d = nc.dram_tensor("xv", [CTXP, D2], F16, kind="ExternalInput").ap()
    mT_d = nc.dram_tensor("mT", [D2, D2], F16, kind="ExternalInput").ap()
    wb_d = nc.dram_tensor("wb", [SH, 192], F16, kind="ExternalOutput").ap()
    rs_d = nc.dram_tensor("rs", [128, NBLK], F32, kind="ExternalOutput").ap()
    ro_d = nc.dram_tensor("ro", [SH, D2], F16, kind="ExternalOutput").ap()

    with tile.TileContext(nc) as tc:
        with (
            tc.tile_pool(name="const", bufs=1) as cp,
            tc.tile_pool(name="spool", bufs=3) as s_pool,
            tc.tile_pool(name="wpool", bufs=3) as w_pool,
            tc.tile_pool(name="wtpool", bufs=3) as wt_pool,
            tc.tile_pool(name="ropool", bufs=3) as ro_pool,
            tc.tile_pool(name="stats", bufs=6) as st_pool,
            tc.tile_pool(name="pa", bufs=2, space="PSUM") as pa,
            tc.tile_pool(name="ps", bufs=2, space="PSUM") as ps,
            tc.tile_pool(name="pt", bufs=2, space="PSUM") as pt,
            tc.tile_pool(name="pr", bufs=2, space="PSUM") as pr,
            ExitStack() as loop_ctx,
        ):
            ident = cp.tile([128, 128], F16)
            make_identity(nc, ident[:])
            # band mask: allowed iff qi <= kj <= qi+63, else NEG (additive)
            ms_sb = cp.tile([128, WIN], F32)
            nc.gpsimd.memset(ms_sb[:], 0.0)
            nc.gpsimd.affine_select(
                out=ms_sb[:], in_=ms_sb[:], compare_op=mybir.AluOpType.is_ge,
                fill=NEG, base=0, pattern=[[1, WIN]], channel_multiplier=-1,
            )
            nc.gpsimd.affine_select(
                out=ms_sb[:], in_=ms_sb[:], compare_op=mybir.AluOpType.is_ge,
                fill=NEG, base=BAND - 1, pattern=[[-1, WIN]], channel_multiplier=1,
            )
            xT_sb = cp.tile([128, 4, CTXP], F16)
            xv_sb = cp.tile([128, CTXP // 128, D2], F16)
            mT_sb = cp.tile([128, 4, D2], F16)
            zt_sb = cp.tile([128, 4, SH], F16)
            rs_sb = cp.tile([128, NBLK], F32)

            if not do_io:
                # compute-only probe: one-time zero-init so matmul inputs
                # have writers (outside the bench loop)
                for t in (xT_sb, mT_sb, xv_sb):
                    nc.gpsimd.memset(t[:], 0.0)

            if reps > 1:
                loop_ctx.enter_context(tc.For_i(0, reps, 1, staggered_reset=True))

            # ---- input loads (inside the bench loop so per-iteration time
            # includes the real input DMA cost) ----
            if do_io:
                for dc in range(4):
                    nc.sync.dma_start(
                        mT_sb[:, dc, :], mT_d[dc * 128 : (dc + 1) * 128, :]
                    )
                for dc in range(4):
                    for c0, c1 in XT_CHUNKS:
                        nc.sync.dma_start(
                            xT_sb[:, dc, c0:c1],
                            xT_d[dc * 128 : (dc + 1) * 128, c0:c1],
                        )
                for t in range(CTXP // 128):
                    nc.sync.dma_start(xv_sb[:, t, :], xv_d[t * 128 : (t + 1) * 128, :])

            if do_compute:
                # ---- phase A: zT = (Wq^T Wk / sqrt(d2))^T-proj of queries ----
                for st in range(4):
                    for ec in range(4):
                        pz = pa.tile([128, 512], F32, tag="pz", name="pz")
                        for dc in range(4):
                            nc.tensor.matmul(
                                pz[:],
                                mT_sb[:, dc, ec * 128 : (ec + 1) * 128],
                                xT_sb[:, dc, BAND + st * 512 : BAND + (st + 1) * 512],
                                start=(dc == 0),
                                stop=(dc == 3),
                            )
                        dst = zt_sb[:, ec, st * 512 : (st + 1) * 512]
                        if (st * 4 + ec) % 2 == 0:
                            nc.scalar.copy(dst, pz[:])
                        else:
                            nc.vector.tensor_copy(dst, pz[:])

            # ---- phase B: per 128-query block ----
            for p in range(NBLK):
                if do_compute:
                    sps = ps.tile([128, WIN], F32, tag="sps", name="sps")
                    for cc in range(4):
                        nc.tensor.matmul(
                            sps[:],
                            zt_sb[:, cc, p * 128 : (p + 1) * 128],
                            xT_sb[:, cc, p * 128 : p * 128 + WIN],
                            start=(cc == 0),
                            stop=(cc == 3),
                        )
                    s_sb = s_pool.tile([128, WIN], F32, tag="s", name="s_sb")
                    nc.vector.tensor_add(s_sb[:], sps[:], ms_sb[:])
                    w_sb = w_pool.tile([128, WIN], F16, tag="w", name="w_sb")
                    nc.scalar.activation(
                        w_sb[:], s_sb[:], Exp, accum_out=rs_sb[:, p : p + 1]
                    )
                if do_io:
                    wb_src = w_sb[:] if do_compute else xT_sb[:, 0, 0:192]
                    nc.sync.dma_start(wb_d[p * 128 : (p + 1) * 128, :], wb_src)

                if do_compute:
                    radd = st_pool.tile([128, 1], F32, tag="radd", name="radd")
                    nc.vector.tensor_scalar_add(radd[:], rs_sb[:, p : p + 1], 1e-30)
                    rinv = st_pool.tile([128, 1], F32, tag="rinv", name="rinv")
                    nc.vector.reciprocal(rinv[:], radd[:])

                    tps = pt.tile([128, 2, 128], F16, tag="tps", name="tps")
                    nc.tensor.transpose(tps[:, 0, :], w_sb[:, 0:128], ident[:])
                    nc.tensor.transpose(tps[0:64, 1, :], w_sb[:, 128:192], ident[:])
                    wt_sb = wt_pool.tile([128, 2, 128], F16, tag="wt", name="wt_sb")
                    nc.vector.tensor_copy(wt_sb[:, 0, :], tps[:, 0, :])
                    nc.vector.tensor_copy(wt_sb[0:64, 1, :], tps[0:64, 1, :])

                    rps = pr.tile([128, D2], F32, tag="rps", name="rps")
                    nc.tensor.matmul(
                        rps[:], wt_sb[:, 0, :], xv_sb[:, p, :], start=True, stop=False
                    )
                    nc.tensor.matmul(
                        rps[:],
                        wt_sb[0:64, 1, :],
                        xv_sb[0:64, p + 1, :],
                        start=False,
                        stop=True,
                    )
                    ro_sb = ro_pool.tile([128, D2], F16, tag="ro", name="ro_sb")
                    nc.scalar.activation(ro_sb[:], rps[:], Copy, scale=rinv[:])
                if do_io:
                    ro_src = ro_sb[:] if do_compute else xT_sb[:, 0, 0:D2]
                    nc.sync.dma_start(ro_d[p * 128 : (p + 1) * 128, :], ro_src)

            if do_io:
                rs_src = rs_sb[:] if do_compute else ms_sb[:, 0:NBLK]
                nc.sync.dma_start(rs_d, rs_src)

    nc.compile()
    return nc


def _get_program(reps: int = 1, probe: str | None = None):
    key = (reps, probe)
    if key not in _PROGRAMS:
        _PROGRAMS[key] = _build_program(reps, probe)
    return _PROGRAMS[key]


def _make_in_maps(states, Wq, Wk):
    x = np.ascontiguousarray(states.reshape(B, S, D2), dtype=np.float32)
    scale = np.float64(D2) ** -0.5
    mT = ((Wq.T.astype(np.float64) @ Wk.astype(np.float64)) * scale).astype(np.float32)

    in_maps = []
    for c in range(8):
        b, h = c // 2, c % 2
        s0 = h * SH
        xpad = np.zeros((S + 2 * BAND, D2), dtype=np.float32)
        xpad[BAND : BAND + S] = x[b]
        xv = np.ascontiguousarray(xpad[s0 : s0 + CTXP])
        xT = np.ascontiguousarray(xv.T)
        in_maps.append(
            {
                "xT": xT.astype(np.float16),
                "xv": xv.astype(np.float16),
                "mT": mT.astype(np.float16),
            }
        )
    return in_maps


def _assemble(res):
    retrieved = np.empty((B, S, D2), dtype=np.float32)
    w = np.zeros((B, S, S), dtype=np.float32)
    edge = (BAND - np.arange(BAND)).astype(np.float32)  # spurious exp(0) count
    for c in range(8):
        b, h = c // 2, c % 2
        s0 = h * SH
        r = res[c]
        ro = r["ro"].astype(np.float32)
        rs = r["rs"].T.reshape(SH).astype(np.float32)  # [qi, p] -> row p*128+qi
        rs_used = rs.copy()
        if h == 0:
            # zero-padded keys contributed exp(0)=1 each to rows 0..63
            rs_used[:BAND] = rs[:BAND] - edge
            fix = rs[:BAND] / np.maximum(rs_used[:BAND], 1e-30)
            ro[:BAND] *= fix[:, None]
        retrieved[b, s0 : s0 + SH] = ro
        wn = r["wb"].astype(np.float32) / np.maximum(rs_used, 1e-30)[:, None]
        for p in range(NBLK):
            g0 = s0 + p * 128
            c0 = g0 - BAND
            blk = wn[p * 128 : (p + 1) * 128]
            if c0 < 0:
                w[b, g0 : g0 + 128, 0 : c0 + 192] = blk[:, -c0:]
            else:
                w[b, g0 : g0 + 128, c0 : c0 + 192] = blk
    retrieved[:, 0, :] = 0.0
    w[:, 0, :] = 0.0
    return retrieved.reshape(B, S, D2 // 2, 2), w


def kernel(states: np.ndarray, Wq: np.ndarray, Wk: np.ndarray):
    from concourse.bass_utils import run_bass_kernel_spmd

    in_maps = _make_in_maps(states, Wq, Wk)
    nc = _get_program()
    res = run_bass_kernel_spmd(nc, in_maps, core_ids=list(range(8))).results
    return _assemble(res)


# revision 21
# speedup vs baseline: 2.7817x; 1.5839x over previous
"""Trainium2 Bass kernel for banded (episodic-memory) attention.

Module computation (B=4, S=4096, D=256, d2=512, band width 64):
    x = states.reshape(B, S, 512)
    q = x @ Wq.T ; k = x @ Wk.T
    scores = q @ k.T / sqrt(512), masked to j in [i-64, i-1]
    w = softmax(scores)  (fully-masked row 0 -> 0)
    retrieved = w @ x
    returns (retrieved.reshape(B,S,256,2), w)

Device strategy (8 cores = 4 batches x 2 sequence halves):
    scores[i,j] = x_i^T (Wq^T Wk) x_j / sqrt(512) = z_i . x_j with
    z_i = (Wq^T Wk)^T x_i -- one fused projection instead of two, and the
    key side of the score matmul is x^T directly.  Per 128-query block the
    live key window is 256 wide (band is 64), so scores are a [128, 256]
    tile with a static additive band mask (built on-device); exp with
    fused row-sum (no max-subtract needed: |s| is a few units at most);
    retrieval contracts the 256-window transposed weights against the
    values.  Unnormalized band + row sums go back to the host, which
    normalizes and scatters into the dense [S, S] output (entries outside
    the band are exactly 0).  Score-path matmuls run in float32r
    (TF32-like, ~1.5e-4 rel err, full PE rate at N>=256); the value path
    runs in bf16.  The left sequence edge (h=0) is handled by zero-padded
    keys: padded columns contribute exp(0)=1 to the row sum, which the
    host subtracts exactly before normalizing.
"""

import sys
from contextlib import ExitStack

if "/opt/trn_rl_repo" not in sys.path:
    sys.path.insert(0, "/opt/trn_rl_repo")

import numpy as np

B, S, D2 = 4, 4096, 512
BAND = 64
SH = S // 2          # 2048 rows per core
CTXP = SH + 2 * BAND  # 2176 padded context rows (= 17 * 128)
NBLK = SH // 128      # 16 query blocks per core
WIN = 192             # key window per query block (banded: 64 + 128)
NEG = -1e30
# xT column-chunk boundaries: chunk t exactly covers phase-A tile t's needs
XT_CHUNKS = [(0, 576), (576, 1088), (1088, 1600), (1600, 2176)]

_PROGRAMS = {}


def _build_program(reps: int = 1, probe: str | None = None):
    # reps > 1 repeats the whole compute body (same inputs/outputs) inside an
    # on-device For_i loop so a benchmark can difference wall times to
    # isolate per-iteration device time.
    # probe: None (full kernel) | "io" (DMAs only) | "compute" (no DMAs).
    do_io = probe != "compute"
    do_compute = probe != "io"
    import concourse.tile as tile
    from concourse import bacc, mybir
    from concourse.masks import make_identity

    F32 = mybir.dt.float32
    F32R = mybir.dt.float32r
    F16 = mybir.dt.float16
    Exp = mybir.ActivationFunctionType.Exp
    Copy = mybir.ActivationFunctionType.Copy

    nc = bacc.Bacc("TRN2", target_bir_lowering=False, debug=False)

    xT_d = nc.dram_tensor("xT", [D2, CTXP], F16, kind="ExternalInput").ap()
    xv_d = nc.dram_tensor("xv", [CTXP, D2], F16, kind="ExternalInput").ap()
    mT_d = nc.dram_tensor("mT", [D2, D2], F16, kind="ExternalInput").ap()
    wb_d = nc.dram_tensor("wb", [SH, 192], F16, kind="ExternalOutput").ap()
    rs_d = nc.dram_tensor("rs", [128, NBLK], F32, kind="ExternalOutput").ap()
    ro_d = nc.dram_tensor("ro", [SH, D2], F16, kind="ExternalOutput").ap()

    with tile.TileContext(nc) as tc:
        with (
            tc.tile_pool(name="const", bufs=1) as cp,
            tc.tile_pool(name="spool", bufs=3) as s_pool,
            tc.tile_pool(name="wpool", bufs=3) as w_pool,
            tc.tile_pool(name="wtpool", bufs=3) as wt_pool,
            tc.tile_pool(name="ropool", bufs=3) as ro_pool,
            tc.tile_pool(name="stats", bufs=6) as st_pool,
            tc.tile_pool(name="pa", bufs=2, space="PSUM") as pa,
            tc.tile_pool(name="ps", bufs=2, space="PSUM") as ps,
            tc.tile_pool(name="pt", bufs=2, space="PSUM") as pt,
            tc.tile_pool(name="pr", bufs=2, space="PSUM") as pr,
            ExitStack() as loop_ctx,
        ):
            ident = cp.tile([128, 128], F16)
            make_identity(nc, ident[:])
            # band mask: allowed iff qi <= kj <= qi+63, else NEG (additive)
            ms_sb = cp.tile([128, WIN], F32)
            nc.gpsimd.memset(ms_sb[:], 0.0)
            nc.gpsimd.affine_select(
                out=ms_sb[:], in_=ms_sb[:], compare_op=mybir.AluOpType.is_ge,
                fill=NEG, base=0, pattern=[[1, WIN]], channel_multiplier=-1,
            )
            nc.gpsimd.affine_select(
                out=ms_sb[:], in_=ms_sb[:], compare_op=mybir.AluOpType.is_ge,
                fill=NEG, base=BAND - 1, pattern=[[-1, WIN]], channel_multiplier=1,
            )
            xT_sb = cp.tile([128, 4, CTXP], F16)
            xv_sb = cp.tile([128, CTXP // 128, D2], F16)
            mT_sb = cp.tile([128, 4, D2], F16)
            zt_sb = cp.tile([128, 4, SH], F16)
            rs_sb = cp.tile([128, NBLK], F32)
            wb_sb = cp.tile([128, NBLK, WIN], F16)
            ro_st = cp.tile([128, NBLK, D2], F16)
            wb_r = wb_d.rearrange("(t p) c -> p t c", p=128)
            ro_r = ro_d.rearrange("(t p) d -> p t d", p=128)

            if not do_io:
                # compute-only probe: one-time zero-init so matmul inputs
                # have writers (outside the bench loop)
                for t in (xT_sb, mT_sb, xv_sb):
                    nc.gpsimd.memset(t[:], 0.0)

            if reps > 1:
                loop_ctx.enter_context(tc.For_i(0, reps, 1, staggered_reset=True))

            # ---- input loads (inside the bench loop so per-iteration time
            # includes the real input DMA cost) ----
            if do_io:
                mT_r = mT_d.rearrange("(dc p) e -> p dc e", p=128)
                nc.sync.dma_start(mT_sb[:], mT_r)
                for dc in range(4):
                    nc.sync.dma_start(
                        xT_sb[:, dc, :], xT_d[dc * 128 : (dc + 1) * 128, :]
                    )
                xv_r = xv_d.rearrange("(t p) d -> p t d", p=128)
                for t0, t1 in ((0, 4), (4, 8), (8, 12), (12, 17)):
                    nc.sync.dma_start(xv_sb[:, t0:t1, :], xv_r[:, t0:t1, :])

            if do_compute:
                # ---- phase A: zT = (Wq^T Wk / sqrt(d2))^T-proj of queries ----
                for st in range(4):
                    for ec in range(4):
                        pz = pa.tile([128, 512], F32, tag="pz", name="pz")
                        for dc in range(4):
                            nc.tensor.matmul(
                                pz[:],
                                mT_sb[:, dc, ec * 128 : (ec + 1) * 128],
                                xT_sb[:, dc, BAND + st * 512 : BAND + (st + 1) * 512],
                                start=(dc == 0),
                                stop=(dc == 3),
                            )
                        dst = zt_sb[:, ec, st * 512 : (st + 1) * 512]
                        if (st * 4 + ec) % 2 == 0:
                            nc.scalar.copy(dst, pz[:])
                        else:
                            nc.vector.tensor_copy(dst, pz[:])

            # ---- phase B: per 128-query block ----
            for p in range(NBLK):
                if do_compute:
                    sps = ps.tile([128, WIN], F32, tag="sps", name="sps")
                    for cc in range(4):
                        nc.tensor.matmul(
                            sps[:],
                            zt_sb[:, cc, p * 128 : (p + 1) * 128],
                            xT_sb[:, cc, p * 128 : p * 128 + WIN],
                            start=(cc == 0),
                            stop=(cc == 3),
                        )
                    s_sb = s_pool.tile([128, WIN], F32, tag="s", name="s_sb")
                    nc.vector.tensor_add(s_sb[:], sps[:], ms_sb[:])
                    w_sb = wb_sb[:, p, :]
                    nc.scalar.activation(
                        w_sb, s_sb[:], Exp, accum_out=rs_sb[:, p : p + 1]
                    )

                if do_compute:
                    radd = st_pool.tile([128, 1], F32, tag="radd", name="radd")
                    nc.vector.tensor_scalar_add(radd[:], rs_sb[:, p : p + 1], 1e-30)
                    rinv = st_pool.tile([128, 1], F32, tag="rinv", name="rinv")
                    nc.vector.reciprocal(rinv[:], radd[:])

                    tps = pt.tile([128, 2, 128], F16, tag="tps", name="tps")
                    nc.tensor.transpose(tps[:, 0, :], w_sb[:, 0:128], ident[:])
                    nc.tensor.transpose(tps[0:64, 1, :], w_sb[:, 128:192], ident[:])
                    wt_sb = wt_pool.tile([128, 2, 128], F16, tag="wt", name="wt_sb")
                    nc.vector.tensor_copy(wt_sb[:, 0, :], tps[:, 0, :])
                    nc.vector.tensor_copy(wt_sb[0:64, 1, :], tps[0:64, 1, :])

                    rps = pr.tile([128, D2], F32, tag="rps", name="rps")
                    nc.tensor.matmul(
                        rps[:], wt_sb[:, 0, :], xv_sb[:, p, :], start=True, stop=False
                    )
                    nc.tensor.matmul(
                        rps[:],
                        wt_sb[0:64, 1, :],
                        xv_sb[0:64, p + 1, :],
                        start=False,
                        stop=True,
                    )
                    nc.scalar.activation(ro_st[:, p, :], rps[:], Copy, scale=rinv[:])
                if do_io and p % 4 == 3:
                    # flush staged outputs in big strided DMAs
                    g0 = p - 3
                    if do_compute:
                        nc.sync.dma_start(ro_r[:, g0 : p + 1, :], ro_st[:, g0 : p + 1, :])
                    else:
                        nc.sync.dma_start(ro_r[:, g0 : p + 1, :], xv_sb[:, g0 : p + 1, :])
                    if p % 8 == 7:
                        if do_compute:
                            nc.sync.dma_start(
                                wb_r[:, p - 7 : p + 1, :], wb_sb[:, p - 7 : p + 1, :]
                            )
                        else:
                            nc.sync.dma_start(
                                wb_r[:, p - 7 : p + 1, :],
                                xv_sb[:, p - 7 : p + 1, 0:WIN],
                            )

            if do_io:
                rs_src = rs_sb[:] if do_compute else ms_sb[:, 0:NBLK]
                nc.sync.dma_start(rs_d, rs_src)

    nc.compile()
    return nc


def _get_program(reps: int = 1, probe: str | None = None):
    key = (reps, probe)
    if key not in _PROGRAMS:
        _PROGRAMS[key] = _build_program(reps, probe)
    return _PROGRAMS[key]


def _make_in_maps(states, Wq, Wk):
    x = np.ascontiguousarray(states.reshape(B, S, D2), dtype=np.float32)
    scale = np.float64(D2) ** -0.5
    mT = ((Wq.T.astype(np.float64) @ Wk.astype(np.float64)) * scale).astype(np.float32)

    in_maps = []
    for c in range(8):
        b, h = c // 2, c % 2
        s0 = h * SH
        xpad = np.zeros((S + 2 * BAND, D2), dtype=np.float32)
        xpad[BAND : BAND + S] = x[b]
        xv = np.ascontiguousarray(xpad[s0 : s0 + CTXP])
        xT = np.ascontiguousarray(xv.T)
        in_maps.append(
            {
                "xT": xT.astype(np.float16),
                "xv": xv.astype(np.float16),
                "mT": mT.astype(np.float16),
            }
        )
    return in_maps


def _assemble(res):
    retrieved = np.empty((B, S, D2), dtype=np.float32)
    w = np.zeros((B, S, S), dtype=np.float32)
    edge = (BAND - np.arange(BAND)).astype(np.float32)  # spurious exp(0) count
    for c in range(8):
        b, h = c // 2, c % 2
        s0 = h * SH
        r = res[c]
        ro = r["ro"].astype(np.float32)
        rs = r["rs"].T.reshape(SH).astype(np.float32)  # [qi, p] -> row p*128+qi
        rs_used = rs.copy()
        if h == 0:
            # zero-padded keys contributed exp(0)=1 each to rows 0..63
            rs_used[:BAND] = rs[:BAND] - edge
            fix = rs[:BAND] / np.maximum(rs_used[:BAND], 1e-30)
            ro[:BAND] *= fix[:, None]
        retrieved[b, s0 : s0 + SH] = ro
        wn = r["wb"].astype(np.float32) / np.maximum(rs_used, 1e-30)[:, None]
        for p in range(NBLK):
            g0 = s0 + p * 128
            c0 = g0 - BAND
            blk = wn[p * 128 : (p + 1) * 128]
            if c0 < 0:
                w[b, g0 : g0 + 128, 0 : c0 + 192] = blk[:, -c0:]
            else:
                w[b, g0 : g0 + 128, c0 : c0 + 192] = blk
    retrieved[:, 0, :] = 0.0
    w[:, 0, :] = 0.0
    return retrieved.reshape(B, S, D2 // 2, 2), w


def kernel(states: np.ndarray, Wq: np.ndarray, Wk: np.ndarray):
    from concourse.bass_utils import run_bass_kernel_spmd

    in_maps = _make_in_maps(states, Wq, Wk)
    nc = _get_program()
    res = run_bass_kernel_spmd(nc, in_maps, core_ids=list(range(8))).results
    return _assemble(res)
